# revision 1
# baseline (speedup 1.0000x reference)
"""Sliding-window GQA causal self-attention for Trainium2, 8 NeuronCores.

Sharding: 8 cores = 4 batches x 2 head-shards. Each core handles one batch
and 2 of the 4 KV groups (8 of 16 Q heads). Core computes a full [C, T]
partial of the output projection; host sums the two shards per batch.

On-core layouts (T = 1024 tokens of one batch):
  xt   [C, T]          x^T, contraction operand for all projections
  qTf  4 x [128, T]    roped+rms'd q^T; tile r rows = [head(g0,r) 64d ; head(g1,r) 64d]
  kTf  [128, T]        roped k^T (rms folded into the exp scale)
  v    [128, 8, 128]   v natural, v[p, j, c] = v[t=128j+p, ch], ch = 64*gg + d
  probs^T per (r, gg, kblock j): [128 kpos, <=384 qpos], band-masked exp(scores^T)
  y^T  4 x [128, T]    attention out, same row layout as qTf
  outT [C, T]          partial output projection (host sums shard pair, transposes)

All matmul operands are float32r (tf32-like, ~2e-4 rel err, full PE rate at
moving-dim >= 256).
"""
import numpy as np

B, T, C = 4, 1024, 1024
H, HKV, D = 16, 4, 64
REP = H // HKV
WINDOW = 256
GATE_CH = 12
NCORES = 8
EPS = float(np.finfo(np.float32).eps)
QK_SCALE = 1.2 * 1.2 / 8.0  # the two rms scales (1.2 each) * 1/sqrt(D)

_CACHE = {}


def _build_program(debug=False, reps=1):
    from contextlib import ExitStack
    import concourse.bass as bass
    import concourse.tile as tile
    from concourse import bacc, mybir
    from concourse.masks import make_identity

    f32 = mybir.dt.float32
    f32r = mybir.dt.float32r
    ts = bass.ts

    nc = bacc.Bacc("TRN2", target_bir_lowering=False, debug=False,
                   enable_asserts=True, num_devices=NCORES)

    def din(name, shape, dt=f32):
        return nc.dram_tensor(name, shape, dt, kind="ExternalInput").ap()

    xt = din("xt", [C, T], f32r)
    wq = din("wq", [C, 512], f32r)
    wk = din("wk", [C, 128], f32r)
    wv = din("wv", [C, 128], f32r)
    wo = din("wo", [512, C], f32r)
    wg = din("wg", [16, 2], f32r)        # zero-padded from 12 gate channels
    vet = din("vet", [128, T])           # 3 * ve^T rows [g0 64d ; g1 64d]
    cosb = din("cosb", [128, T])
    sinbw = din("sinbw", [128, T])       # swap32(sinb): u = z*sinbw, t2 = swap(u)
    indq8 = din("indq8", [128, 4, 8], f32r)  # [:, r, 2r+gg] = 1/64 (block rows)
    indqn = din("indqn", [128, 2], f32r)     # block indicator / 64
    indb = din("indb", [2, 128], f32r)       # block-broadcast rows, value 1
    ind018 = din("ind018", [8, 4, 128], f32r)  # [2r+gg, r, m]=QK_SCALE, gg=m//64
    onesg = din("onesg", [128, 2, 2], f32r)  # [:, gg, gg] = 1 else 0
    epsb = din("epsb", [128, 1])             # rms epsilon
    outT = nc.dram_tensor("out_t", [C, T], f32, kind="ExternalOutput").ap()
    dbg = {}
    if debug:
        for nm, shp in [("d_qTf", [512, T]), ("d_kTf", [128, T]),
                        ("d_v", [128, 8, 128]),
                        ("d_rsq", [8, T]), ("d_yTf", [512, T])]:
            dbg[nm] = nc.dram_tensor(nm, shp, f32, kind="ExternalOutput").ap()

    Exp = mybir.ActivationFunctionType.Exp
    Sqrt = mybir.ActivationFunctionType.Sqrt
    Sigmoid = mybir.ActivationFunctionType.Sigmoid
    Square = mybir.ActivationFunctionType.Square
    Copy = mybir.ActivationFunctionType.Copy
    is_ge = mybir.AluOpType.is_ge

    def rope_swap(dst, src):
        # dst[p] = src[p +/- 32] within each 64-row head block
        nc.sync.dma_start(dst[0:32, :], src[32:64, :])
        nc.sync.dma_start(dst[32:64, :], src[0:32, :])
        nc.sync.dma_start(dst[64:96, :], src[96:128, :])
        nc.sync.dma_start(dst[96:128, :], src[64:96, :])

    with tile.TileContext(nc) as tc:
     for _rep in range(reps):
      with ExitStack() as ctx:
        sing = ctx.enter_context(tc.tile_pool(name="sing", bufs=1))

        # ---------- persistent tiles ----------
        wo_sb = sing.tile([128, 4, C], f32r, name="wo_sb")
        for kr in range(4):
            nc.sync.dma_start(wo_sb[:, kr, :], wo[ts(kr, 128), :])
        indq8_sb = sing.tile([128, 4, 8], f32r, name="indq8_sb")
        nc.sync.dma_start(indq8_sb[:], indq8[:])
        indqn_sb = sing.tile([128, 2], f32r, name="indqn_sb")
        nc.sync.dma_start(indqn_sb[:], indqn[:])
        indb_sb = sing.tile([2, 128], f32r, name="indb_sb")
        nc.sync.dma_start(indb_sb[:], indb[:])
        ind018_sb = sing.tile([8, 4, 128], f32r, name="ind018_sb")
        nc.sync.dma_start(ind018_sb[:], ind018[:])
        onesg_sb = sing.tile([128, 2, 2], f32r, name="onesg_sb")
        nc.sync.dma_start(onesg_sb[:], onesg[:])
        epsb_sb = sing.tile([128, 1], f32, name="epsb_sb")
        nc.sync.dma_start(epsb_sb[:], epsb[:])
        ident = sing.tile([128, 128], f32, name="ident")
        make_identity(nc, ident[:])

        qTf = [sing.tile([128, T], f32r, name=f"qTf{r}") for r in range(4)]
        kTf = sing.tile([128, T], f32r, name="kTf")
        v_sb = sing.tile([128, 8, 128], f32r, name="v_sb")
        yTf = [sing.tile([128, T], f32r, name=f"yTf{r}") for r in range(4)]
        rsq_sb = sing.tile([8, T], f32r, name="rsq_sb")

        # ================= Stage A: projections / rope / rms / gate =========
        with tc.tile_pool(name="stA", bufs=2) as stA, \
             tc.tile_pool(name="pA_", bufs=1, space="PSUM") as pA_:
            xt_sb = stA.tile([128, 8, T], f32r, name="xt_sb", bufs=1)
            for kc in range(8):
                nc.sync.dma_start(xt_sb[:, kc, :], xt[ts(kc, 128), :])
            wq_sb = stA.tile([128, 8, 512], f32r, name="wq_sb", bufs=1)
            wk_sb = stA.tile([128, 8, 128], f32r, name="wk_sb", bufs=1)
            wv_sb = stA.tile([128, 8, 128], f32r, name="wv_sb", bufs=1)
            for kc in range(8):
                nc.sync.dma_start(wq_sb[:, kc, :], wq[ts(kc, 128), :])
                nc.sync.dma_start(wk_sb[:, kc, :], wk[ts(kc, 128), :])
                nc.sync.dma_start(wv_sb[:, kc, :], wv[ts(kc, 128), :])
            wg_sb = stA.tile([16, 2], f32r, name="wg_sb", bufs=1)
            nc.sync.dma_start(wg_sb[:], wg[:])
            vet_sb = stA.tile([128, T], f32, name="vet_sb", bufs=1)
            nc.sync.dma_start(vet_sb[:], vet[:])
            cosb_sb = stA.tile([128, T], f32, name="cosb_sb", bufs=1)
            nc.sync.dma_start(cosb_sb[:], cosb[:])
            sinbw_sb = stA.tile([128, T], f32, name="sinbw_sb", bufs=1)
            nc.sync.dma_start(sinbw_sb[:], sinbw[:])

            for h in range(2):
                tsl = slice(512 * h, 512 * h + 512)

                # ---- projections, streamed over xt chunks: k, v, q0, q1 first
                k_ps = pA_.tile([128, 512], f32, name="k_ps", tag="kps")
                v_ps = pA_.tile([128, 512], f32, name="v_ps", tag="vps")
                q_ps01 = [pA_.tile([128, 512], f32, name=f"q_ps{r}", tag="qps",
                                   bufs=2) for r in range(2)]
                for kc in range(8):
                    st, sp = kc == 0, kc == 7
                    nc.tensor.matmul(k_ps[:], wk_sb[:, kc, :], xt_sb[:, kc, tsl],
                                     start=st, stop=sp)
                    nc.tensor.matmul(v_ps[:], wv_sb[:, kc, :], xt_sb[:, kc, tsl],
                                     start=st, stop=sp)
                    for r in range(2):
                        nc.tensor.matmul(q_ps01[r][:], wq_sb[:, kc, ts(r, 128)],
                                         xt_sb[:, kc, tsl], start=st, stop=sp)
                g_ps = pA_.tile([2, 512], f32, name="g_ps", tag="mix")
                nc.tensor.matmul(g_ps[:], wg_sb[:], xt_sb[0:16, 0, tsl],
                                 start=True, stop=True)

                # ---- gate + value-embedding; v' = v + (3*sigmoid(g)) * ve
                sig_sb = stA.tile([2, 512], f32r, name="sig_sb", tag="sig")
                nc.scalar.activation(sig_sb[:], g_ps[:], Sigmoid)
                gb_ps = pA_.tile([128, 512], f32, name="gb_ps", tag="mix")
                nc.tensor.matmul(gb_ps[:], indb_sb[:], sig_sb[:],
                                 start=True, stop=True)
                gve_sb = stA.tile([128, 512], f32, name="gve_sb", tag="gve")
                nc.vector.tensor_mul(gve_sb[:], gb_ps[:], vet_sb[:, tsl])
                vp_sb = stA.tile([128, 512], f32, name="vp_sb", tag="vp")
                nc.vector.tensor_add(vp_sb[:], v_ps[:], gve_sb[:])
                for tb in range(4):
                    vt_ps = pA_.tile([128, 128], f32, name="vt_ps", tag="mix")
                    nc.tensor.transpose(vt_ps[:], vp_sb[:, ts(tb, 128)], ident[:])
                    nc.vector.tensor_copy(v_sb[:, 4 * h + tb, :], vt_ps[:])

                # ---- k: rope into kTf, then fold rstd_k into kTf
                ku_sb = stA.tile([128, 512], f32, name="ku_sb", tag="sw")
                nc.vector.tensor_mul(ku_sb[:], k_ps[:], sinbw_sb[:, tsl])
                ksw_sb = stA.tile([128, 512], f32, name="ksw_sb", tag="sw2")
                rope_swap(ksw_sb, ku_sb)
                nc.vector.tensor_mul(kTf[:, tsl], k_ps[:], cosb_sb[:, tsl])
                nc.vector.tensor_add(kTf[:, tsl], kTf[:, tsl], ksw_sb[:])
                k2_sb = stA.tile([128, 512], f32r, name="k2_sb", tag="sq2")
                nc.vector.tensor_mul(k2_sb[:], kTf[:, tsl], kTf[:, tsl])
                mskr_ps = pA_.tile([2, 512], f32, name="mskr_ps", tag="mskr")
                nc.tensor.matmul(mskr_ps[:], indqn_sb[:], k2_sb[:],
                                 start=True, stop=True)
                sk1 = stA.tile([2, 512], f32, name="sk1", tag="sk1")
                nc.scalar.activation(sk1[:], mskr_ps[:], Sqrt,
                                     bias=epsb_sb[0:2, :])
                rk_sb = stA.tile([2, 512], f32r, name="rk_sb", tag="rk")
                with nc.allow_low_precision("f32r rstd_k"):
                    nc.vector.reciprocal(rk_sb[:], sk1[:])
                rkb_ps = pA_.tile([128, 512], f32, name="rkb_ps", tag="mix")
                nc.tensor.matmul(rkb_ps[:], indb_sb[:], rk_sb[:],
                                 start=True, stop=True)
                nc.vector.tensor_mul(kTf[:, tsl], kTf[:, tsl], rkb_ps[:])

                # ---- q: rope into qTf + mean-square, r = 0,1 then 2,3
                msq_ps = pA_.tile([8, 512], f32, name="msq_ps", tag="msq",
                                  bufs=2)

                def do_q_rope(r, q_ps_r):
                    qu_sb = stA.tile([128, 512], f32, name="qu_sb", tag="sw")
                    nc.vector.tensor_mul(qu_sb[:], q_ps_r[:], sinbw_sb[:, tsl])
                    qsw_sb = stA.tile([128, 512], f32, name="qsw_sb", tag="sw2")
                    rope_swap(qsw_sb, qu_sb)
                    nc.vector.tensor_mul(qTf[r][:, tsl], q_ps_r[:],
                                         cosb_sb[:, tsl])
                    nc.vector.tensor_add(qTf[r][:, tsl], qTf[r][:, tsl],
                                         qsw_sb[:])
                    q2_sb = stA.tile([128, 512], f32r, name="q2_sb", tag="sq2")
                    nc.scalar.activation(q2_sb[:], qTf[r][:, tsl], Square)
                    nc.tensor.matmul(msq_ps[0:8, :], indq8_sb[:, r, :], q2_sb[:],
                                     start=(r == 0), stop=(r == 3),
                                     skip_group_check=True)

                for r in range(2):
                    do_q_rope(r, q_ps01[r])
                q_ps23 = [pA_.tile([128, 512], f32, name=f"q_ps{r}", tag="qps",
                                   bufs=2) for r in (2, 3)]
                for kc in range(8):
                    for i, r in enumerate((2, 3)):
                        nc.tensor.matmul(q_ps23[i][:], wq_sb[:, kc, ts(r, 128)],
                                         xt_sb[:, kc, tsl], start=(kc == 0),
                                         stop=(kc == 7))
                for i, r in enumerate((2, 3)):
                    do_q_rope(r, q_ps23[i])

                # ---- rstd(q) for this half, then apply rms to q in place
                sq1 = stA.tile([8, 512], f32, name="sq1", tag="sq1")
                nc.scalar.activation(sq1[:], msq_ps[:], Sqrt, bias=epsb_sb[0:8, :])
                with nc.allow_low_precision("f32r rstd"):
                    nc.vector.reciprocal(rsq_sb[:, tsl], sq1[:])
                for r in range(4):
                    rb_ps = pA_.tile([128, 512], f32, name="rb_ps", tag="mix")
                    nc.tensor.matmul(rb_ps[:], ind018_sb[:, r, :], rsq_sb[:, tsl],
                                     start=True, stop=True)
                    nc.vector.tensor_mul(qTf[r][:, tsl], qTf[r][:, tsl], rb_ps[:])


        # ================= Stage B: attention ================================
        with tc.tile_pool(name="stB", bufs=2) as stB, \
             tc.tile_pool(name="pB_", bufs=1, space="PSUM") as pB_:
            for r in range(4):
                pkeep = {j: stB.tile([128, 2, 384], f32r, name=f"pk{j}",
                                     tag=f"pk{j}", bufs=2) for j in (2, 3)}
                for h in range(2):
                    tsl = slice(512 * h, 512 * h + 512)
                    y_ps = [pB_.tile([64, 512], f32, name=f"y_ps{gg}",
                                     tag=f"yps{gg}", bufs=1) for gg in range(2)]
                    sums_ps = pB_.tile([2, 512], f32, name="sums_ps", tag="sums")
                    jlist = list(range(0, 4)) if h == 0 else list(range(2, 8))
                    first = True
                    for j in jlist:
                        w = min(384, T - 128 * j)
                        if h == 1 and j in pkeep:
                            p2 = pkeep[j]          # cached from h == 0
                        else:
                            if j in pkeep:
                                p2 = pkeep[j]
                            else:
                                p2 = stB.tile([128, 2, 384], f32r, name="p2",
                                              tag="p2", bufs=3)
                            sc2 = pB_.tile([128, 2, 512], f32, name="sc2",
                                           tag="sc", bufs=2)
                            nc.tensor.matmul(sc2[:, 0, 0:w],
                                             kTf[0:64, ts(j, 128)],
                                             qTf[r][0:64, 128 * j:128 * j + w],
                                             start=True, stop=True)
                            nc.tensor.matmul(sc2[:, 1, 0:w],
                                             kTf[64:128, ts(j, 128)],
                                             qTf[r][64:128, 128 * j:128 * j + w],
                                             start=True, stop=True)
                            nc.scalar.activation(p2[:, :, 0:w], sc2[:, :, 0:w],
                                                 Exp)
                            wl = min(256, w)
                            # keep cols [0, wl) where i - p >= 0 (causal edge)
                            nc.gpsimd.affine_select(
                                p2[:, :, 0:wl], p2[:, :, 0:wl], compare_op=is_ge,
                                fill=0.0, base=0, pattern=[[0, 2], [1, wl]],
                                channel_multiplier=-1)
                            if w > 256:
                                # keep cols [256, w) where p - i' >= 0 (window)
                                nc.gpsimd.affine_select(
                                    p2[:, :, 256:w], p2[:, :, 256:w],
                                    compare_op=is_ge, fill=0.0, base=0,
                                    pattern=[[0, 2], [-1, w - 256]],
                                    channel_multiplier=1)
                        a = max(128 * j, 512 * h)
                        b = min(128 * j + w, 512 * h + 512)
                        n0, nn = a - 128 * j, b - a
                        for gg in range(2):
                            nc.tensor.matmul(
                                y_ps[gg][:, a - 512 * h:b - 512 * h],
                                v_sb[:, j, ts(gg, 64)], p2[:, gg, n0:n0 + nn],
                                start=first, stop=(j == jlist[-1]),
                                skip_group_check=True)
                            nc.tensor.matmul(
                                sums_ps[:, a - 512 * h:b - 512 * h],
                                onesg_sb[:, gg, :], p2[:, gg, n0:n0 + nn],
                                start=(first and gg == 0),
                                stop=(j == jlist[-1] and gg == 1),
                                skip_group_check=True)
                        first = False
                    # normalize y by 1/sums (broadcast over 64 d rows per head)
                    rsum_sb = stB.tile([2, 512], f32r, name="rsum_sb",
                                       tag="rsum")
                    with nc.allow_low_precision("f32r 1/sums"):
                        nc.vector.reciprocal(rsum_sb[:], sums_ps[:])
                    rbs_ps = pB_.tile([128, 512], f32, name="rbs_ps", tag="rbs")
                    nc.tensor.matmul(rbs_ps[:], indb_sb[:], rsum_sb[:],
                                     start=True, stop=True)
                    rbs_sb = stB.tile([128, 512], f32, name="rbs_sb",
                                      tag="rbs_sb")
                    if h == 0:
                        nc.scalar.activation(rbs_sb[:], rbs_ps[:], Copy)
                    else:
                        nc.vector.tensor_copy(rbs_sb[:], rbs_ps[:])
                    for gg in range(2):
                        nc.vector.tensor_mul(yTf[r][ts(gg, 64), tsl],
                                             y_ps[gg][:], rbs_sb[ts(gg, 64), :])


        # ================= Stage C: output projection ========================
        with tc.tile_pool(name="stC", bufs=3) as stC, \
             tc.tile_pool(name="pC_", bufs=2, space="PSUM") as pC_:
            for ct in range(8):
                for h in range(2):
                    tsl = slice(512 * h, 512 * h + 512)
                    o_ps = pC_.tile([128, 512], f32, name="o_ps", tag="ops")
                    for kr in range(4):
                        nc.tensor.matmul(o_ps[:], wo_sb[:, kr, ts(ct, 128)],
                                         yTf[kr][:, tsl], start=(kr == 0),
                                         stop=(kr == 3))
                    o_sb = stC.tile([128, 512], f32, name="o_sb", tag="osb")
                    if (ct + h) % 2 == 0:
                        nc.vector.tensor_copy(o_sb[:], o_ps[:])
                    else:
                        nc.scalar.activation(o_sb[:], o_ps[:], Copy)
                    nc.sync.dma_start(outT[ts(ct, 128), tsl], o_sb[:])

    nc.compile()
    return nc


def _const_inputs():
    indq8 = np.zeros((128, 4, 8), dtype=np.float32)
    for r in range(4):
        indq8[0:64, r, 2 * r] = 1.0 / D
        indq8[64:128, r, 2 * r + 1] = 1.0 / D
    indqn = np.zeros((128, 2), dtype=np.float32)
    indqn[0:64, 0] = 1.0 / D
    indqn[64:128, 1] = 1.0 / D
    indb = np.zeros((2, 128), dtype=np.float32)
    indb[0, 0:64] = 1.0
    indb[1, 64:128] = 1.0
    ind018 = np.zeros((8, 4, 128), dtype=np.float32)
    for r in range(4):
        ind018[2 * r, r, 0:64] = QK_SCALE
        ind018[2 * r + 1, r, 64:128] = QK_SCALE
    onesg = np.zeros((128, 2, 2), dtype=np.float32)
    onesg[:, 0, 0] = 1.0
    onesg[:, 1, 1] = 1.0
    epsb = np.full((128, 1), EPS, dtype=np.float32)
    return dict(indq8=indq8, indqn=indqn, indb=indb, ind018=ind018,
                onesg=onesg, epsb=epsb)


def _prep_core_inputs(x, ve3, cosb, sinb, sinbw, Wq, Wk, Wv, Wo, Wg, consts, b, s):
    g0, g1 = 2 * s, 2 * s + 1
    xt = np.ascontiguousarray(x[b].T)

    Wq4 = Wq.reshape(HKV, REP, D, C)
    wq_rows = np.concatenate([Wq4[g, r] for r in range(REP) for g in (g0, g1)],
                             axis=0)                       # (512, C)
    wq = np.ascontiguousarray(wq_rows.T)                   # (C, 512)
    Wk3 = Wk.reshape(HKV, D, C)
    wk = np.ascontiguousarray(np.concatenate([Wk3[g0], Wk3[g1]], axis=0).T)
    Wv3 = Wv.reshape(HKV, D, C)
    wv = np.ascontiguousarray(np.concatenate([Wv3[g0], Wv3[g1]], axis=0).T)

    Wo4 = Wo.reshape(C, HKV, REP, D)
    wo_cols = np.concatenate([Wo4[:, g, r, :] for r in range(REP)
                              for g in (g0, g1)], axis=1)  # (C, 512)
    wo = np.ascontiguousarray(wo_cols.T)                   # (512, C)

    wg = np.zeros((16, 2), dtype=np.float32)
    wg[0:GATE_CH, 0] = Wg[g0]
    wg[0:GATE_CH, 1] = Wg[g1]

    ve4 = ve3[b].reshape(T, HKV, D)
    vet = np.ascontiguousarray(
        np.concatenate([ve4[:, g0, :], ve4[:, g1, :]], axis=1).T)  # (128, T)

    d = dict(xt=xt, wq=wq, wk=wk, wv=wv, wo=wo, wg=wg, vet=vet,
             cosb=cosb, sinbw=sinbw)
    d.update(consts)
    return d


def kernel(x, ve, cos, sin, Wq, Wk, Wv, Wo, Wg, window_size):
    from concourse.bass_utils import run_bass_kernel_spmd

    assert int(window_size) == WINDOW
    x = np.asarray(x, dtype=np.float32)
    ve = np.asarray(ve, dtype=np.float32)
    Wq = np.asarray(Wq, dtype=np.float32)
    Wk = np.asarray(Wk, dtype=np.float32)
    Wv = np.asarray(Wv, dtype=np.float32)
    Wo = np.asarray(Wo, dtype=np.float32)
    Wg = np.asarray(Wg, dtype=np.float32)
    c = np.asarray(cos, dtype=np.float32).reshape(T, D // 2)   # (T, 32)
    sn = np.asarray(sin, dtype=np.float32).reshape(T, D // 2)

    cosb = np.ascontiguousarray(np.tile(c.T, (4, 1)))          # (128, T)
    sinb = np.ascontiguousarray(
        np.concatenate([sn.T, -sn.T, sn.T, -sn.T], axis=0))    # (128, T)
    sinbw = np.ascontiguousarray(
        np.concatenate([-sn.T, sn.T, -sn.T, sn.T], axis=0))    # swap32 rows
    ve3 = 3.0 * ve
    consts = _const_inputs()

    if "nc" not in _CACHE:
        _CACHE["nc"] = _build_program()
    nc = _CACHE["nc"]

    in_maps = []
    for core in range(NCORES):
        b, s = core // 2, core % 2
        in_maps.append(_prep_core_inputs(x, ve3, cosb, sinb, sinbw,
                                         Wq, Wk, Wv, Wo, Wg, consts, b, s))

    res = run_bass_kernel_spmd(nc, in_maps, core_ids=list(range(NCORES)))
    out = np.empty((B, T, C), dtype=np.float32)
    for b in range(B):
        acc = res.results[2 * b]["out_t"] + res.results[2 * b + 1]["out_t"]
        out[b] = acc.T
    return out



# revision 12
# speedup vs baseline: 1.0865x; 1.0865x over previous
"""Sliding-window GQA causal self-attention for Trainium2, 8 NeuronCores.

Sharding: 8 cores = 4 batches x 2 head-shards. Each core handles one batch
and 2 of the 4 KV groups (8 of 16 Q heads). Core computes a full [C, T]
partial of the output projection in bf16; host sums the two shards per batch.

All matmul operands are bf16 (rel err ~4e-3 total). Band masks are applied
by seeding the score PSUM region with -1e30 via identity matmuls (same
accumulation group as the scores). Softmax denominators come from a ones
column appended to the V stationary. RoPE's partition swap is a permutation
matmul; rsqrt is exp(-0.5*ln(x)) so the Act engine needs one table only.
"""
import numpy as np
import ml_dtypes

B, T, C = 4, 1024, 1024
H, HKV, D = 16, 4, 64
REP = H // HKV
WINDOW = 256
GATE_CH = 12
NCORES = 8
EPS = float(np.finfo(np.float32).eps)
QK_SCALE = 1.2 * 1.2 / 8.0
NEG = -1.0e30
BF16 = ml_dtypes.bfloat16

_CACHE = {}


def _build_program(debug=False, reps=1):
    from contextlib import ExitStack
    import concourse.bass as bass
    import concourse.tile as tile
    from concourse import bacc, mybir

    f32 = mybir.dt.float32
    bf16 = mybir.dt.bfloat16
    ts = bass.ts

    nc = bacc.Bacc("TRN2", target_bir_lowering=False, debug=False,
                   enable_asserts=True, num_devices=NCORES)

    def din(name, shape, dt=bf16):
        return nc.dram_tensor(name, shape, dt, kind="ExternalInput").ap()

    xt = din("xt", [C, T])
    wq = din("wq", [C, 512])
    wk = din("wk", [C, 128])
    wv = din("wv", [C, 128])
    wo = din("wo", [512, C])
    wgn = din("wgn", [16, 2])            # negated gate weights (16 = padded)
    vet = din("vet", [128, T])           # 3 * ve^T rows [g0 64d ; g1 64d]
    cosb = din("cosb", [128, T])
    sinbw = din("sinbw", [128, T])       # swap32(sin) with sign pattern
    cb16 = din("cb16", [128, 12, 128])   # consts, see _const_inputs
    cones = din("cones", [1, 512])
    cepsb = din("cepsb", [128, 1], f32)
    outT = nc.dram_tensor("out_t", [C, T], bf16, kind="ExternalOutput").ap()

    Exp = mybir.ActivationFunctionType.Exp
    Ln = mybir.ActivationFunctionType.Ln
    Copy = mybir.ActivationFunctionType.Copy
    mult = mybir.AluOpType.mult
    divide = mybir.AluOpType.divide

    with tile.TileContext(nc) as tc:
     for _rep in range(reps):
      with ExitStack() as ctx:
        sing = ctx.enter_context(tc.tile_pool(name="sing", bufs=1))

        # ---------- persistent tiles + input DMAs (priority order) ----------
        cb = sing.tile([128, 12, 128], bf16, name="cb")
        nc.sync.dma_start(cb[:], cb16[:])
        wk_sb = sing.tile([128, 8, 128], bf16, name="wk_sb")
        nc.sync.dma_start(wk_sb[:], wk[:])
        xt_sb = sing.tile([128, 8, T], bf16, name="xt_sb")
        nc.sync.dma_start(xt_sb[:, 0:2, :], xt[0:256, :])
        wv_sb = sing.tile([128, 8, 128], bf16, name="wv_sb")
        nc.sync.dma_start(wv_sb[:], wv[:])
        nc.sync.dma_start(xt_sb[:, 2:4, :], xt[256:512, :])
        wq_sb = sing.tile([128, 8, 512], bf16, name="wq_sb")
        nc.sync.dma_start(wq_sb[:], wq[:])
        nc.sync.dma_start(xt_sb[:, 4:6, :], xt[512:768, :])
        nc.sync.dma_start(xt_sb[:, 6:8, :], xt[768:1024, :])
        wg_sb = sing.tile([16, 2], bf16, name="wg_sb")
        nc.sync.dma_start(wg_sb[:], wgn[:])
        ones512 = sing.tile([1, 512], bf16, name="ones512")
        nc.sync.dma_start(ones512[:], cones[:])
        epsb_sb = sing.tile([128, 1], f32, name="epsb_sb")
        nc.sync.dma_start(epsb_sb[:], cepsb[:])
        cos_sb = sing.tile([128, T], bf16, name="cos_sb")
        nc.sync.dma_start(cos_sb[:], cosb[:])
        sinw_sb = sing.tile([128, T], bf16, name="sinw_sb")
        nc.sync.dma_start(sinw_sb[:], sinbw[:])
        vet_sb = sing.tile([128, T], bf16, name="vet_sb")
        nc.sync.dma_start(vet_sb[:], vet[:])
        wo_sb = sing.tile([128, 4, C], bf16, name="wo_sb")
        nc.sync.dma_start(wo_sb[:], wo[:])

        ident = cb[:, 0, :]
        pswap = cb[:, 1, :]
        tmc = cb[:, 2, :]
        tmw = cb[:, 3, :]
        indq8 = [cb[:, 4, 8 * r:8 * r + 8] for r in range(4)]
        indq2 = cb[:, 4, 32:34]
        ind018 = [cb[0:8, 5 + r, :] for r in range(4)]
        indbk = cb[0:2, 9, :]
        indbg = cb[0:2, 10, :]
        ones128 = cb[0:1, 11, :]

        kTf = sing.tile([128, T], bf16, name="kTf")
        qTf = [sing.tile([128, T], bf16, name=f"qTf{r}") for r in range(4)]
        v_sb = sing.tile([128, 8, 130], bf16, name="v_sb")
        yTf = [sing.tile([128, T], bf16, name=f"yTf{r}") for r in range(4)]
        nc.vector.memset(v_sb[:, :, 64:65], 1.0)
        nc.vector.memset(v_sb[:, :, 129:130], 1.0)

        # ================= Stage A: projections / rope / rms / gate =========
        with tc.tile_pool(name="stA", bufs=2) as stA, \
             tc.tile_pool(name="pA1", bufs=1, space="PSUM") as pA1:

            k_ps = [pA1.tile([128, 512], f32, name=f"k_ps{h}", tag=f"kps{h}")
                    for h in range(2)]
            v_ps = [pA1.tile([128, 512], f32, name=f"v_ps{h}", tag=f"vps{h}")
                    for h in range(2)]
            g_ps = [pA1.tile([2, 512], f32, name=f"g_ps{h}", tag="gps",
                             bufs=1) for h in range(2)]

            def tsl(h):
                return slice(512 * h, 512 * h + 512)

            # --- PE: k, g, v projections
            for h in range(2):
                for kc in range(8):
                    nc.tensor.matmul(k_ps[h][:], wk_sb[:, kc, :],
                                     xt_sb[:, kc, tsl(h)],
                                     start=(kc == 0), stop=(kc == 7))
                nc.tensor.matmul(g_ps[h][:], wg_sb[:], xt_sb[0:16, 0, tsl(h)],
                                 start=True, stop=True)
            for h in range(2):
                for kc in range(8):
                    nc.tensor.matmul(v_ps[h][:], wv_sb[:, kc, :],
                                     xt_sb[:, kc, tsl(h)],
                                     start=(kc == 0), stop=(kc == 7))

            # --- elementwise chains for k and v (other engines)
            k_sb, ku, kc_, k2 = {}, {}, {}, {}
            for h in range(2):
                k_sb[h] = stA.tile([128, 512], bf16, name="k_sb", tag="ksb",
                                   bufs=2)
                nc.scalar.activation(k_sb[h][:], k_ps[h][:], Copy)
                ku[h] = stA.tile([128, 512], bf16, name="ku", tag="ku", bufs=2)
                nc.gpsimd.tensor_mul(ku[h][:], k_sb[h][:], sinw_sb[:, tsl(h)])
                kc_[h] = stA.tile([128, 512], bf16, name="kc_", tag="kc",
                                  bufs=2)
                nc.gpsimd.tensor_mul(kc_[h][:], k_sb[h][:], cos_sb[:, tsl(h)])
                k2[h] = stA.tile([128, 512], bf16, name="k2", tag="k2",
                                 bufs=2)
                nc.vector.tensor_mul(k2[h][:], k_sb[h][:], k_sb[h][:])

            expg = {}
            for h in range(2):
                expg[h] = stA.tile([2, 512], bf16, name="expg", tag="expg",
                                   bufs=2)
                nc.scalar.activation(expg[h][:], g_ps[h][:], Exp)

            # --- PE: gate broadcast (+1 via ones row), rope swaps, k-ms
            gb_ps, ksw_ps, msk_ps = {}, {}, {}
            for h in range(2):
                gb_ps[h] = pA1.tile([128, 512], f32, name="gb_ps", tag="gb",
                                    bufs=1)
                nc.tensor.matmul(gb_ps[h][:], indbg[:], expg[h][:],
                                 start=True, stop=False)
                nc.tensor.matmul(gb_ps[h][:], ones128[:], ones512[:],
                                 start=False, stop=True, skip_group_check=True)
            for h in range(2):
                ksw_ps[h] = pA1.tile([128, 512], f32, name="ksw_ps",
                                     tag="ksw", bufs=1)
                nc.tensor.matmul(ksw_ps[h][:], pswap[:], ku[h][:],
                                 start=True, stop=True)

            # --- v gate & rope-k elementwise
            vp = {}
            for h in range(2):
                gve = stA.tile([128, 512], bf16, name="gve", tag="gve",
                               bufs=2)
                nc.vector.scalar_tensor_tensor(gve[:], vet_sb[:, tsl(h)], 1.0,
                                               gb_ps[h][:], mult, divide)
                vp[h] = stA.tile([128, 512], bf16, name="vp", tag="vp",
                                 bufs=2)
                nc.vector.tensor_add(vp[h][:], v_ps[h][:], gve[:])

            kpre = {}
            for h in range(2):
                kpre[h] = stA.tile([128, 512], bf16, name="kpre", tag="kpre",
                                   bufs=2)
                nc.vector.tensor_add(kpre[h][:], ksw_ps[h][:], kc_[h][:])

            # --- PE: v transposes (all 8 into one single-bank psum tile)
            vt8 = pA1.tile([128, 8, 128], bf16, name="vt8", tag="vt", bufs=1)
            for h in range(2):
                for tb in range(4):
                    nc.tensor.transpose(vt8[:, 4 * h + tb, :],
                                        vp[h][:, ts(tb, 128)], ident[:])
            for h in range(2):
                for tb in range(4):
                    nc.vector.tensor_copy(v_sb[:, 4 * h + tb, 0:64],
                                          vt8[:, 4 * h + tb, 0:64])
                    nc.vector.tensor_copy(v_sb[:, 4 * h + tb, 65:129],
                                          vt8[:, 4 * h + tb, 64:128])

        with tc.tile_pool(name="stA2", bufs=2) as stA, \
             tc.tile_pool(name="pA2", bufs=1, space="PSUM") as pA2:
            def tsl(h):
                return slice(512 * h, 512 * h + 512)

            # --- k ms / rms broadcast / fold (PE + Act + DVE)
            rkb_ps, rk_sb = {}, {}
            for h in range(2):
                msk_ps = pA2.tile([2, 512], f32, name="msk_ps", tag="msk",
                                  bufs=1)
                nc.tensor.matmul(msk_ps[:], indq2[:], k2[h][:],
                                 start=True, stop=True)
                lk = stA.tile([2, 512], f32, name="lk", tag="lk", bufs=2)
                nc.scalar.activation(lk[:], msk_ps[:], Ln,
                                     bias=epsb_sb[0:2, :])
                rk_sb[h] = stA.tile([2, 512], bf16, name="rk_sb", tag="rk",
                                    bufs=2)
                nc.scalar.activation(rk_sb[h][:], lk[:], Exp, scale=-0.5)
            for h in range(2):
                rkb_ps[h] = pA2.tile([128, 512], f32, name="rkb_ps",
                                     tag="rkb", bufs=1)
                nc.tensor.matmul(rkb_ps[h][:], indbk[:], rk_sb[h][:],
                                 start=True, stop=True)
                nc.vector.tensor_mul(kTf[:, tsl(h)], kpre[h][:],
                                     rkb_ps[h][:])

            # --- q projections + rope + rms (r-major)
            q_ps, msq_ps = {}, {}
            for h in range(2):
                msq_ps[h] = pA2.tile([8, 512], f32, name=f"msq{h}",
                                     tag=f"msq{h}")

            q_sb, qu, qc_, q2, qsw_ps = {}, {}, {}, {}, {}

            def q_elem(r, h):
                q_sb = stA.tile([128, 512], bf16, name="q_sb", tag="qsb",
                                bufs=2)
                nc.scalar.activation(q_sb[:], q_ps[(r, h)][:], Copy)
                u = stA.tile([128, 512], bf16, name="qu", tag="qu", bufs=2)
                nc.gpsimd.tensor_mul(u[:], q_sb[:], sinw_sb[:, tsl(h)])
                qu[(r, h)] = u
                c2 = stA.tile([128, 512], bf16, name="qc_", tag="qc", bufs=2)
                nc.gpsimd.tensor_mul(c2[:], q_sb[:], cos_sb[:, tsl(h)])
                qc_[(r, h)] = c2
                s2 = stA.tile([128, 512], bf16, name="q2", tag="q2", bufs=2)
                nc.vector.tensor_mul(s2[:], q_sb[:], q_sb[:])
                q2[(r, h)] = s2

            for r in range(4):
                for h in range(2):
                    qp = pA2.tile([128, 512], f32, name=f"q_ps{r}{h}",
                                  tag="qps", bufs=2)
                    for kc in range(8):
                        nc.tensor.matmul(qp[:], wq_sb[:, kc, ts(r, 128)],
                                         xt_sb[:, kc, tsl(h)],
                                         start=(kc == 0), stop=(kc == 7))
                    q_ps[(r, h)] = qp
                    q_elem(r, h)
                # PE: swap + msq for this r (elementwise of r runs meanwhile)
                for h in range(2):
                    sw = pA2.tile([128, 512], f32, name="qsw_ps", tag="qsw",
                                  bufs=1)
                    nc.tensor.matmul(sw[:], pswap[:], qu[(r, h)][:],
                                     start=True, stop=True)
                    qsw_ps[(r, h)] = sw
                    nc.tensor.matmul(msq_ps[h][:], indq8[r][:],
                                     q2[(r, h)][:], start=(r == 0),
                                     stop=(r == 3), skip_group_check=True)
                for h in range(2):
                    qpre = stA.tile([128, 512], bf16, name="qpre", tag="qpre",
                                    bufs=4)
                    nc.vector.tensor_add(qpre[:], qsw_ps[(r, h)][:],
                                         qc_[(r, h)][:])
                    q_sb, qu[(r, h)] = None, None
                    q2[(r, h)] = None
                    qc_[(r, h)] = qpre   # reuse slot to stash qpre

            rsq_sb = {}
            for h in range(2):
                lq = stA.tile([8, 512], f32, name="lq", tag="lq", bufs=2)
                nc.scalar.activation(lq[:], msq_ps[h][:], Ln,
                                     bias=epsb_sb[0:8, :])
                rsq_sb[h] = stA.tile([8, 512], bf16, name="rsq", tag="rsq",
                                     bufs=2)
                nc.scalar.activation(rsq_sb[h][:], lq[:], Exp, scale=-0.5)
            for r in range(4):
                for h in range(2):
                    rb_ps = pA2.tile([128, 512], f32, name="rb_ps", tag="rb",
                                     bufs=1)
                    nc.tensor.matmul(rb_ps[:], ind018[r][:], rsq_sb[h][:],
                                     start=True, stop=True)
                    nc.vector.tensor_mul(qTf[r][:, tsl(h)], qc_[(r, h)][:],
                                         rb_ps[:])

        # ================= Stage B: attention ================================
        with tc.tile_pool(name="stB", bufs=2) as stB, \
             tc.tile_pool(name="pB_", bufs=1, space="PSUM") as pB_:
            for r in range(4):
                pkeep = {j: stB.tile([128, 2, 3, 128], bf16, name=f"pk{j}",
                                     tag=f"pk{j}", bufs=2) for j in (2, 3)}
                for h in range(2):
                    hsl = slice(512 * h, 512 * h + 512)
                    y_ps = pB_.tile([65, 2, 512], f32, name="y_ps",
                                    tag="yps", bufs=2)
                    jlist = list(range(0, 4)) if h == 0 else list(range(2, 8))
                    first = True
                    for j in jlist:
                        w = min(384, T - 128 * j)
                        ns = w // 128          # number of 128-wide segments
                        fresh = not (h == 1 and j in (2, 3))
                        if not fresh:
                            p2 = pkeep[j]      # cached from h == 0
                        else:
                            p2 = pkeep.get(j)
                            if p2 is None:
                                p2 = stB.tile([128, 2, 3, 128], bf16,
                                              name="p2", tag="p2", bufs=3)
                            sc2 = pB_.tile([128, 2, 3, 128], f32, name="sc2",
                                           tag="sc", bufs=2)
                            for gg in range(2):
                                dsl = slice(64 * gg, 64 * gg + 64)
                                qsl0 = slice(128 * j, 128 * j + 128)
                                # diag segment: mask seed + score, one group
                                nc.tensor.matmul(
                                    sc2[:, gg, 0, :], ident, tmc,
                                    start=True, stop=False)
                                nc.tensor.matmul(
                                    sc2[:, gg, 0, :], kTf[dsl, qsl0],
                                    qTf[r][dsl, qsl0], start=False, stop=True,
                                    skip_group_check=True)
                                if ns > 1:    # mid segment: no mask
                                    qsl1 = slice(128 * j + 128, 128 * j + 256)
                                    nc.tensor.matmul(
                                        sc2[:, gg, 1, :],
                                        kTf[dsl, qsl0], qTf[r][dsl, qsl1],
                                        start=True, stop=True,
                                        skip_group_check=True)
                                if ns > 2:    # window segment: seed + score
                                    qsl2 = slice(128 * j + 256, 128 * j + 384)
                                    nc.tensor.matmul(
                                        sc2[:, gg, 2, :], ident, tmw,
                                        start=True, stop=False,
                                        skip_group_check=True)
                                    nc.tensor.matmul(
                                        sc2[:, gg, 2, :], kTf[dsl, qsl0],
                                        qTf[r][dsl, qsl2], start=False,
                                        stop=True, skip_group_check=True)
                            nc.scalar.activation(p2[:, :, 0:ns, :],
                                                 sc2[:, :, 0:ns, :], Exp)
                        a = max(128 * j, 512 * h)
                        b = min(128 * j + w, 512 * h + 512)
                        s0, s1 = (a - 128 * j) // 128, (b - 128 * j) // 128
                        for gg in range(2):
                            nc.tensor.matmul(
                                y_ps[:, gg, a - 512 * h:b - 512 * h],
                                v_sb[:, j, 65 * gg:65 * gg + 65],
                                p2[:, gg, s0:s1, :],
                                start=first, stop=(j == jlist[-1]),
                                skip_group_check=True)
                        first = False
                    # normalize: 1/sums, broadcast via ones matmul, 2 muls
                    rsum = stB.tile([1, 2, 512], bf16, name="rsum",
                                    tag="rsum", bufs=2)
                    with nc.allow_low_precision("1/sums bf16"):
                        nc.vector.reciprocal(rsum[:], y_ps[64:65, :, :])
                    rbs_ps = pB_.tile([128, 512], f32, name="rbs_ps",
                                      tag="sc", bufs=2)
                    nc.tensor.matmul(rbs_ps[0:64, :], ones128[:, 0:64],
                                     rsum[:, 0, :], start=True, stop=True)
                    nc.tensor.matmul(rbs_ps[64:128, :], ones128[:, 0:64],
                                     rsum[:, 1, :], start=True, stop=True,
                                     skip_group_check=True)
                    rbs_sb = stB.tile([128, 512], bf16, name="rbs_sb",
                                      tag="rbs_sb", bufs=2)
                    nc.scalar.activation(rbs_sb[:], rbs_ps[:], Copy)
                    for gg in range(2):
                        nc.vector.tensor_mul(yTf[r][ts(gg, 64), hsl],
                                             y_ps[0:64, gg, :],
                                             rbs_sb[ts(gg, 64), :])

        # ================= Stage C: output projection ========================
        with tc.tile_pool(name="stC", bufs=3) as stC, \
             tc.tile_pool(name="pC_", bufs=2, space="PSUM") as pC_:
            for h in range(2):
                hsl = slice(512 * h, 512 * h + 512)
                for ct in range(8):
                    o_ps = pC_.tile([128, 512], f32, name="o_ps", tag="ops")
                    for kr in range(4):
                        nc.tensor.matmul(o_ps[:], wo_sb[:, kr, ts(ct, 128)],
                                         yTf[kr][:, hsl], start=(kr == 0),
                                         stop=(kr == 3))
                    o_sb = stC.tile([128, 512], bf16, name="o_sb", tag="osb")
                    if ct % 2 == 0:
                        nc.vector.tensor_copy(o_sb[:], o_ps[:])
                    else:
                        nc.scalar.activation(o_sb[:], o_ps[:], Copy)
                    nc.sync.dma_start(outT[ts(ct, 128), hsl], o_sb[:])

    nc.compile()
    return nc


def _const_inputs():
    cb = np.zeros((128, 12, 128), dtype=np.float32)
    # 0: identity
    cb[:, 0, :] = np.eye(128, dtype=np.float32)
    # 1: pswap  P[c, m] = 1 iff c == swap(m), swap = +-32 within 64-block
    m = np.arange(128)
    sw = np.where((m % 64) < 32, m + 32, m - 32)
    cb[sw, 1, m] = 1.0
    # 2: Tc diag mask (keep qcol >= kpos), 3: Tw window mask (keep qcol <= kpos)
    p = np.arange(128)[:, None]
    c = np.arange(128)[None, :]
    cb[:, 2, :] = np.where(c >= p, 0.0, NEG)
    cb[:, 3, :] = np.where(c <= p, 0.0, NEG)
    # 4: cols 0:32 indq8 flat [128,(4,8)], cols 32:34 indq2
    for r in range(4):
        cb[0:64, 4, 8 * r + 2 * r] = 1.0 / D
        cb[64:128, 4, 8 * r + 2 * r + 1] = 1.0 / D
    cb[0:64, 4, 32] = 1.0 / D
    cb[64:128, 4, 33] = 1.0 / D
    # 5..8: ind018 per r
    for r in range(4):
        cb[2 * r, 5 + r, 0:64] = QK_SCALE
        cb[2 * r + 1, 5 + r, 64:128] = QK_SCALE
    # 9: indbk rows 0:2 (1.2), 10: indbg rows 0:2 (1.0), 11: ones row 0
    cb[0, 9, 0:64] = 1.2
    cb[1, 9, 64:128] = 1.2
    cb[0, 10, 0:64] = 1.0
    cb[1, 10, 64:128] = 1.0
    cb[0, 11, :] = 1.0
    epsb = np.full((128, 1), EPS, dtype=np.float32)
    return dict(cb16=cb.astype(BF16), cepsb=epsb)


def _prep_core_inputs(x, ve3, cosb, sinbw, Wq, Wk, Wv, Wo, Wg, consts, b, s):
    g0, g1 = 2 * s, 2 * s + 1
    bf = lambda a: np.ascontiguousarray(a).astype(BF16)
    xt = bf(x[b].T)

    Wq4 = Wq.reshape(HKV, REP, D, C)
    wq_rows = np.concatenate([Wq4[g, r] for r in range(REP) for g in (g0, g1)],
                             axis=0)                       # (512, C)
    wq = bf(wq_rows.T)                                     # (C, 512)
    Wk3 = Wk.reshape(HKV, D, C)
    wk = bf(np.concatenate([Wk3[g0], Wk3[g1]], axis=0).T)
    Wv3 = Wv.reshape(HKV, D, C)
    wv = bf(np.concatenate([Wv3[g0], Wv3[g1]], axis=0).T)

    Wo4 = Wo.reshape(C, HKV, REP, D)
    wo_cols = np.concatenate([Wo4[:, g, r, :] for r in range(REP)
                              for g in (g0, g1)], axis=1)  # (C, 512)
    wo = bf(wo_cols.T)                                     # (512, C)

    wgn = np.zeros((16, 2), dtype=np.float32)
    wgn[0:GATE_CH, 0] = -Wg[g0]
    wgn[0:GATE_CH, 1] = -Wg[g1]

    ve4 = ve3[b].reshape(T, HKV, D)
    vet = bf(np.concatenate([ve4[:, g0, :], ve4[:, g1, :]], axis=1).T)

    d = dict(xt=xt, wq=wq, wk=wk, wv=wv, wo=wo, wgn=wgn.astype(BF16),
             vet=vet, cosb=cosb, sinbw=sinbw)
    d.update(consts)
    return d


def kernel(x, ve, cos, sin, Wq, Wk, Wv, Wo, Wg, window_size):
    from concourse.bass_utils import run_bass_kernel_spmd

    assert int(window_size) == WINDOW
    x = np.asarray(x, dtype=np.float32)
    ve = np.asarray(ve, dtype=np.float32)
    Wq = np.asarray(Wq, dtype=np.float32)
    Wk = np.asarray(Wk, dtype=np.float32)
    Wv = np.asarray(Wv, dtype=np.float32)
    Wo = np.asarray(Wo, dtype=np.float32)
    Wg = np.asarray(Wg, dtype=np.float32)
    c = np.asarray(cos, dtype=np.float32).reshape(T, D // 2)   # (T, 32)
    sn = np.asarray(sin, dtype=np.float32).reshape(T, D // 2)

    cosb = np.ascontiguousarray(np.tile(c.T, (4, 1))).astype(BF16)
    sinbw = np.ascontiguousarray(
        np.concatenate([-sn.T, sn.T, -sn.T, sn.T], axis=0)).astype(BF16)
    ve3 = 3.0 * ve
    consts = _const_inputs()
    consts["cones"] = np.ones((1, 512), dtype=BF16)

    if "nc" not in _CACHE:
        _CACHE["nc"] = _build_program()
    nc = _CACHE["nc"]

    in_maps = []
    for core in range(NCORES):
        b, s = core // 2, core % 2
        in_maps.append(_prep_core_inputs(x, ve3, cosb, sinbw,
                                         Wq, Wk, Wv, Wo, Wg, consts, b, s))

    res = run_bass_kernel_spmd(nc, in_maps, core_ids=list(range(NCORES)))
    out = np.empty((B, T, C), dtype=np.float32)
    for b in range(B):
        acc = (res.results[2 * b]["out_t"].astype(np.float32)
               + res.results[2 * b + 1]["out_t"].astype(np.float32))
        out[b] = acc.T
    return out


# revision 17
# speedup vs baseline: 1.1412x; 1.0503x over previous
"""Sliding-window GQA causal self-attention for Trainium2, 8 NeuronCores.

Sharding: 8 cores = 4 batches x 2 head-shards. Each core handles one batch
and 2 of the 4 KV groups (8 of 16 Q heads). Core computes a full [C, T]
partial of the output projection in bf16; host sums the two shards per batch.

All matmul operands are bf16 (rel err ~4e-3 total). Band masks are applied
by seeding the score PSUM region with -1e30 via identity matmuls (same
accumulation group as the scores). Softmax denominators come from a ones
column appended to the V stationary. RoPE's partition swap is a permutation
matmul; rsqrt is exp(-0.5*ln(x)) so the Act engine needs one table only.
"""
import numpy as np
import ml_dtypes

B, T, C = 4, 1024, 1024
H, HKV, D = 16, 4, 64
REP = H // HKV
WINDOW = 256
GATE_CH = 12
NCORES = 8
EPS = float(np.finfo(np.float32).eps)
QK_SCALE = 1.2 * 1.2 / 8.0
NEG = -1.0e30
BF16 = ml_dtypes.bfloat16

_CACHE = {}


def _build_program(debug=False, reps=1):
    from contextlib import ExitStack
    import concourse.bass as bass
    import concourse.tile as tile
    from concourse import bacc, mybir

    f32 = mybir.dt.float32
    bf16 = mybir.dt.bfloat16
    ts = bass.ts

    nc = bacc.Bacc("TRN2", target_bir_lowering=False, debug=False,
                   enable_asserts=True, num_devices=NCORES)

    def din(name, shape, dt=bf16):
        return nc.dram_tensor(name, shape, dt, kind="ExternalInput").ap()

    xt = din("xt", [C, T])
    wq = din("wq", [C, 512])
    wk = din("wk", [C, 128])
    wv = din("wv", [C, 128])
    wo = din("wo", [512, C])
    wgn = din("wgn", [16, 2])            # negated gate weights (16 = padded)
    vet = din("vet", [128, T])           # 3 * ve^T rows [g0 64d ; g1 64d]
    cosb = din("cosb", [128, T])
    sinbw = din("sinbw", [128, T])       # swap32(sin) with sign pattern
    cb16 = din("cb16", [128, 12, 128])   # consts, see _const_inputs
    cones = din("cones", [1, 512])
    cepsb = din("cepsb", [128, 1], f32)
    outT = nc.dram_tensor("out_t", [C, T], bf16, kind="ExternalOutput").ap()

    Exp = mybir.ActivationFunctionType.Exp
    Sqrt = mybir.ActivationFunctionType.Sqrt
    Copy = mybir.ActivationFunctionType.Copy
    mult = mybir.AluOpType.mult
    divide = mybir.AluOpType.divide

    with tile.TileContext(nc) as tc:
     for _rep in range(reps):
      with ExitStack() as ctx:
        sing = ctx.enter_context(tc.tile_pool(name="sing", bufs=1))

        # ---------- persistent tiles + input DMAs (priority order) ----------
        cb = sing.tile([128, 12, 128], bf16, name="cb")
        nc.sync.dma_start(cb[:], cb16[:])
        wk_sb = sing.tile([128, 8, 128], bf16, name="wk_sb")
        nc.sync.dma_start(wk_sb[:], wk[:])
        xt_sb = sing.tile([128, 8, T], bf16, name="xt_sb")
        nc.sync.dma_start(xt_sb[:, 0:2, :], xt[0:256, :])
        wv_sb = sing.tile([128, 8, 128], bf16, name="wv_sb")
        nc.sync.dma_start(wv_sb[:], wv[:])
        nc.sync.dma_start(xt_sb[:, 2:4, :], xt[256:512, :])
        wq_sb = sing.tile([128, 8, 512], bf16, name="wq_sb")
        nc.sync.dma_start(wq_sb[:], wq[:])
        nc.sync.dma_start(xt_sb[:, 4:6, :], xt[512:768, :])
        nc.sync.dma_start(xt_sb[:, 6:8, :], xt[768:1024, :])
        wg_sb = sing.tile([16, 2], bf16, name="wg_sb")
        nc.sync.dma_start(wg_sb[:], wgn[:])
        ones512 = sing.tile([1, 512], bf16, name="ones512")
        nc.sync.dma_start(ones512[:], cones[:])
        epsb_sb = sing.tile([128, 1], f32, name="epsb_sb")
        nc.sync.dma_start(epsb_sb[:], cepsb[:])
        cos_sb = sing.tile([128, T], bf16, name="cos_sb")
        nc.sync.dma_start(cos_sb[:], cosb[:])
        sinw_sb = sing.tile([128, T], bf16, name="sinw_sb")
        nc.sync.dma_start(sinw_sb[:], sinbw[:])
        vet_sb = sing.tile([128, T], bf16, name="vet_sb")
        nc.sync.dma_start(vet_sb[:], vet[:])
        wo_sb = sing.tile([128, 4, C], bf16, name="wo_sb")
        nc.sync.dma_start(wo_sb[:], wo[:])

        ident = cb[:, 0, :]
        pswap = cb[:, 1, :]
        tmc = cb[:, 2, :]
        tmw = cb[:, 3, :]
        indq4 = [cb[:, 4, 0:4], cb[:, 4, 4:8]]   # even r / odd r
        ind014 = [cb[0:4, 5 + r, :] for r in range(4)]
        indq2 = cb[:, 4, 32:34]

        indbk = cb[0:2, 9, :]
        indbg = cb[0:2, 10, :]
        ones128 = cb[0:1, 11, :]

        kTf = sing.tile([128, T], bf16, name="kTf")
        qTf = [sing.tile([128, T], bf16, name=f"qTf{r}") for r in range(4)]
        v_sb = sing.tile([128, 8, 130], bf16, name="v_sb")
        yTf = [sing.tile([128, T], bf16, name=f"yTf{r}") for r in range(4)]
        nc.vector.memset(v_sb[:, :, 64:65], 1.0)
        nc.vector.memset(v_sb[:, :, 129:130], 1.0)

        # ================= Stage A: projections / rope / rms / gate =========
        with tc.tile_pool(name="stA", bufs=2) as stA, \
             tc.tile_pool(name="pA1", bufs=1, space="PSUM") as pA1:

            k_ps = [pA1.tile([128, 512], f32, name=f"k_ps{h}", tag=f"kps{h}")
                    for h in range(2)]
            v_ps = [pA1.tile([128, 512], f32, name=f"v_ps{h}", tag=f"vps{h}")
                    for h in range(2)]
            g_ps = [pA1.tile([2, 512], f32, name=f"g_ps{h}", tag="gps",
                             bufs=1) for h in range(2)]

            def tsl(h):
                return slice(512 * h, 512 * h + 512)

            # --- PE pass 1: k, g, v projections, chunk-interleaved with
            # the xt DMA arrivals (2 kc per DMA chunk)
            for kc in range(8):
                for h in range(2):
                    nc.tensor.matmul(k_ps[h][:], wk_sb[:, kc, :],
                                     xt_sb[:, kc, tsl(h)],
                                     start=(kc == 0), stop=(kc == 7))
                if kc == 0:
                    for h in range(2):
                        nc.tensor.matmul(g_ps[h][:], wg_sb[:],
                                         xt_sb[0:16, 0, tsl(h)],
                                         start=True, stop=True)
                for h in range(2):
                    nc.tensor.matmul(v_ps[h][:], wv_sb[:, kc, :],
                                     xt_sb[:, kc, tsl(h)],
                                     start=(kc == 0), stop=(kc == 7))

            # --- elementwise chains for k and v (other engines)
            k_sb, ku, kc_, k2 = {}, {}, {}, {}
            for h in range(2):
                k_sb[h] = stA.tile([128, 512], bf16, name="k_sb", tag="ksb",
                                   bufs=2)
                nc.scalar.activation(k_sb[h][:], k_ps[h][:], Copy)
                ku[h] = stA.tile([128, 512], bf16, name="ku", tag="ku", bufs=2)
                nc.gpsimd.tensor_mul(ku[h][:], k_sb[h][:], sinw_sb[:, tsl(h)])
                kc_[h] = stA.tile([128, 512], bf16, name="kc_", tag="kc",
                                  bufs=2)
                nc.gpsimd.tensor_mul(kc_[h][:], k_sb[h][:], cos_sb[:, tsl(h)])
                k2[h] = sing.tile([128, 512], bf16, name=f"k2_{h}")
                nc.vector.tensor_mul(k2[h][:], k_sb[h][:], k_sb[h][:])

            expg = {}
            for h in range(2):
                expg[h] = stA.tile([2, 512], bf16, name="expg", tag="expg",
                                   bufs=2)
                nc.scalar.activation(expg[h][:], g_ps[h][:], Exp)

            # --- PE: gate broadcast (+1 via ones row), rope swaps
            gb_ps, ksw_ps = {}, {}
            for h in range(2):
                gb_ps[h] = pA1.tile([128, 512], f32, name="gb_ps", tag="gb",
                                    bufs=1)
                nc.tensor.matmul(gb_ps[h][:], indbg[:], expg[h][:],
                                 start=True, stop=False)
                nc.tensor.matmul(gb_ps[h][:], ones128[:], ones512[:],
                                 start=False, stop=True, skip_group_check=True)
            for h in range(2):
                ksw_ps[h] = pA1.tile([128, 512], f32, name="ksw_ps",
                                     tag="ksw", bufs=1)
                nc.tensor.matmul(ksw_ps[h][:], pswap[:], ku[h][:],
                                 start=True, stop=True)

            # --- v gate & rope-k elementwise
            vp = {}
            for h in range(2):
                gve = stA.tile([128, 512], bf16, name="gve", tag="gve",
                               bufs=2)
                nc.vector.scalar_tensor_tensor(gve[:], vet_sb[:, tsl(h)], 1.0,
                                               gb_ps[h][:], mult, divide)
                vp[h] = stA.tile([128, 512], bf16, name="vp", tag="vp",
                                 bufs=2)
                nc.vector.tensor_add(vp[h][:], v_ps[h][:], gve[:])

            kpre = {}
            for h in range(2):
                kpre[h] = sing.tile([128, 512], bf16, name=f"kpre{h}")
                nc.vector.tensor_add(kpre[h][:], ksw_ps[h][:], kc_[h][:])

            # --- PE: v transposes (all 8 into one single-bank psum tile)
            vt8 = pA1.tile([128, 8, 128], bf16, name="vt8", tag="vt", bufs=1)
            for h in range(2):
                for tb in range(4):
                    nc.tensor.transpose(vt8[:, 4 * h + tb, :],
                                        vp[h][:, ts(tb, 128)], ident[:])
            for h in range(2):
                for tb in range(4):
                    nc.vector.tensor_copy(v_sb[:, 4 * h + tb, 0:64],
                                          vt8[:, 4 * h + tb, 0:64])
                    nc.vector.tensor_copy(v_sb[:, 4 * h + tb, 65:129],
                                          vt8[:, 4 * h + tb, 64:128])

        with tc.tile_pool(name="stA2", bufs=2) as stA, \
             tc.tile_pool(name="pA2", bufs=1, space="PSUM") as pA2:
            def tsl(h):
                return slice(512 * h, 512 * h + 512)

            q_ps, msq_ps = {}, {}
            for h in range(2):
                for p in range(2):
                    msq_ps[(h, p)] = pA2.tile([4, 512], f32,
                                              name=f"msq{h}{p}",
                                              tag=f"msq{h}", bufs=1)

            qu, qc_, q2, qsw_ps, qpre = {}, {}, {}, {}, {}

            def q_elem(r, h):
                q_sb = stA.tile([128, 512], bf16, name="q_sb", tag="qsb",
                                bufs=2)
                nc.scalar.activation(q_sb[:], q_ps[(r, h)][:], Copy)
                u = stA.tile([128, 512], bf16, name="qu", tag="qu", bufs=2)
                nc.gpsimd.tensor_mul(u[:], q_sb[:], sinw_sb[:, tsl(h)])
                qu[(r, h)] = u
                c2 = stA.tile([128, 512], bf16, name="qc_", tag="qc", bufs=2)
                nc.gpsimd.tensor_mul(c2[:], q_sb[:], cos_sb[:, tsl(h)])
                qc_[(r, h)] = c2
                s2 = stA.tile([128, 512], bf16, name="q2", tag="q2", bufs=2)
                nc.vector.tensor_mul(s2[:], q_sb[:], q_sb[:])
                q2[(r, h)] = s2

            def q_pe_tail(r):
                # swap matmul + msq accumulation for row-pair of r
                for h in range(2):
                    sw = pA2.tile([128, 512], f32, name="qsw_ps", tag="qsw",
                                  bufs=1)
                    nc.tensor.matmul(sw[:], pswap[:], qu[(r, h)][:],
                                     start=True, stop=True)
                    qsw_ps[(r, h)] = sw
                    nc.tensor.matmul(msq_ps[(h, r // 2)][:], indq4[r % 2][:],
                                     q2[(r, h)][:], start=(r % 2 == 0),
                                     stop=(r % 2 == 1), skip_group_check=True)

            def q_add(r):
                for h in range(2):
                    qp = stA.tile([128, 512], bf16, name="qpre", tag="qpre",
                                  bufs=4)
                    nc.vector.tensor_add(qp[:], qsw_ps[(r, h)][:],
                                         qc_[(r, h)][:])
                    qpre[(r, h)] = qp

            def q_rms(p):
                # sqrt + divide-fold for r pair p (r = 2p, 2p+1)
                for h in range(2):
                    sq4 = stA.tile([4, 512], bf16, name="sq4", tag="sq4",
                                   bufs=2)
                    nc.scalar.activation(sq4[:], msq_ps[(h, p)][:], Sqrt,
                                         bias=epsb_sb[0:4, :])
                    for r in (2 * p, 2 * p + 1):
                        rb_ps = pA2.tile([128, 512], f32, name="rb_ps",
                                         tag="rb", bufs=1)
                        nc.tensor.matmul(rb_ps[:], ind014[r][:], sq4[:],
                                         start=True, stop=True)
                        nc.vector.tensor_tensor(qTf[r][:, tsl(h)],
                                                qpre[(r, h)][:], rb_ps[:],
                                                divide)

            def q_proj(r):
                for h in range(2):
                    qp = pA2.tile([128, 512], f32, name=f"q_ps{r}{h}",
                                  tag="qps", bufs=2)
                    for kc in range(8):
                        nc.tensor.matmul(qp[:], wq_sb[:, kc, ts(r, 128)],
                                         xt_sb[:, kc, tsl(h)],
                                         start=(kc == 0), stop=(kc == 7))
                    q_ps[(r, h)] = qp
                    q_elem(r, h)

            # k-rms (PE parts woven between q projections)
            msk_ps, rkb_ps, sqk = {}, {}, {}
            q_proj(0)
            for h in range(2):
                msk_ps[h] = pA2.tile([2, 512], f32, name="msk_ps", tag="msk",
                                     bufs=1)
                nc.tensor.matmul(msk_ps[h][:], indq2[:], k2[h][:],
                                 start=True, stop=True)
                sqk[h] = stA.tile([2, 512], bf16, name="sqk", tag="sqk",
                                  bufs=2)
                nc.scalar.activation(sqk[h][:], msk_ps[h][:], Sqrt,
                                     bias=epsb_sb[0:2, :])
            q_proj(1)
            q_pe_tail(0)
            for h in range(2):
                rkb_ps[h] = pA2.tile([128, 512], f32, name="rkb_ps",
                                     tag="rkb", bufs=1)
                nc.tensor.matmul(rkb_ps[h][:], indbk[:], sqk[h][:],
                                 start=True, stop=True)
                nc.vector.tensor_tensor(kTf[:, tsl(h)], kpre[h][:],
                                        rkb_ps[h][:], divide)
            q_add(0)
            q_proj(2)
            q_pe_tail(1)
            q_add(1)
            q_rms(0)          # qTf r0, r1 finalized here
            q_proj(3)
            q_pe_tail(2)
            q_add(2)
            q_pe_tail(3)
            q_add(3)
            q_rms(1)

        # ================= Stage B: attention ================================
        with tc.tile_pool(name="stB", bufs=2) as stB, \
             tc.tile_pool(name="pB_", bufs=1, space="PSUM") as pB_:
            for r in range(4):
                pkeep = {j: stB.tile([128, 2, 3, 128], bf16, name=f"pk{j}",
                                     tag=f"pk{j}", bufs=2) for j in (2, 3)}
                for h in range(2):
                    hsl = slice(512 * h, 512 * h + 512)
                    y_ps = pB_.tile([65, 2, 512], f32, name="y_ps",
                                    tag="yps", bufs=2)
                    jlist = list(range(0, 4)) if h == 0 else list(range(2, 8))
                    first = True
                    for j in jlist:
                        w = min(384, T - 128 * j)
                        ns = w // 128          # number of 128-wide segments
                        fresh = not (h == 1 and j in (2, 3))
                        if not fresh:
                            p2 = pkeep[j]      # cached from h == 0
                        else:
                            p2 = pkeep.get(j)
                            if p2 is None:
                                p2 = stB.tile([128, 2, 3, 128], bf16,
                                              name="p2", tag="p2", bufs=3)
                            sc2 = pB_.tile([128, 2, 3, 128], f32, name="sc2",
                                           tag="sc", bufs=2)
                            for gg in range(2):
                                dsl = slice(64 * gg, 64 * gg + 64)
                                qsl0 = slice(128 * j, 128 * j + 128)
                                # diag segment: mask seed + score, one group
                                nc.tensor.matmul(
                                    sc2[:, gg, 0, :], ident, tmc,
                                    start=True, stop=False)
                                nc.tensor.matmul(
                                    sc2[:, gg, 0, :], kTf[dsl, qsl0],
                                    qTf[r][dsl, qsl0], start=False, stop=True,
                                    skip_group_check=True)
                                if ns > 1:    # mid segment: no mask
                                    qsl1 = slice(128 * j + 128, 128 * j + 256)
                                    nc.tensor.matmul(
                                        sc2[:, gg, 1, :],
                                        kTf[dsl, qsl0], qTf[r][dsl, qsl1],
                                        start=True, stop=True,
                                        skip_group_check=True)
                                if ns > 2:    # window segment: seed + score
                                    qsl2 = slice(128 * j + 256, 128 * j + 384)
                                    nc.tensor.matmul(
                                        sc2[:, gg, 2, :], ident, tmw,
                                        start=True, stop=False,
                                        skip_group_check=True)
                                    nc.tensor.matmul(
                                        sc2[:, gg, 2, :], kTf[dsl, qsl0],
                                        qTf[r][dsl, qsl2], start=False,
                                        stop=True, skip_group_check=True)
                            nc.scalar.activation(p2[:, :, 0:ns, :],
                                                 sc2[:, :, 0:ns, :], Exp)
                        a = max(128 * j, 512 * h)
                        b = min(128 * j + w, 512 * h + 512)
                        s0, s1 = (a - 128 * j) // 128, (b - 128 * j) // 128
                        for gg in range(2):
                            nc.tensor.matmul(
                                y_ps[:, gg, a - 512 * h:b - 512 * h],
                                v_sb[:, j, 65 * gg:65 * gg + 65],
                                p2[:, gg, s0:s1, :],
                                start=first, stop=(j == jlist[-1]),
                                skip_group_check=True)
                        first = False
                    # normalize: 1/sums, broadcast via ones matmul, 2 muls
                    rsum = stB.tile([1, 2, 512], bf16, name="rsum",
                                    tag="rsum", bufs=2)
                    with nc.allow_low_precision("1/sums bf16"):
                        nc.vector.reciprocal(rsum[:], y_ps[64:65, :, :])
                    rbs_ps = pB_.tile([128, 512], f32, name="rbs_ps",
                                      tag="sc", bufs=2)
                    nc.tensor.matmul(rbs_ps[0:64, :], ones128[:, 0:64],
                                     rsum[:, 0, :], start=True, stop=True)
                    nc.tensor.matmul(rbs_ps[64:128, :], ones128[:, 0:64],
                                     rsum[:, 1, :], start=True, stop=True,
                                     skip_group_check=True)
                    rbs_sb = stB.tile([128, 512], bf16, name="rbs_sb",
                                      tag="rbs_sb", bufs=2)
                    nc.scalar.activation(rbs_sb[:], rbs_ps[:], Copy)
                    for gg in range(2):
                        nc.vector.tensor_mul(yTf[r][ts(gg, 64), hsl],
                                             y_ps[0:64, gg, :],
                                             rbs_sb[ts(gg, 64), :])

        # ================= Stage C: output projection ========================
        with tc.tile_pool(name="stC", bufs=3) as stC, \
             tc.tile_pool(name="pC_", bufs=2, space="PSUM") as pC_:
            for h in range(2):
                hsl = slice(512 * h, 512 * h + 512)
                for ct in range(8):
                    o_ps = pC_.tile([128, 512], f32, name="o_ps", tag="ops")
                    for kr in range(4):
                        nc.tensor.matmul(o_ps[:], wo_sb[:, kr, ts(ct, 128)],
                                         yTf[kr][:, hsl], start=(kr == 0),
                                         stop=(kr == 3))
                    o_sb = stC.tile([128, 512], bf16, name="o_sb", tag="osb")
                    if ct % 2 == 0:
                        nc.vector.tensor_copy(o_sb[:], o_ps[:])
                    else:
                        nc.scalar.activation(o_sb[:], o_ps[:], Copy)
                    nc.sync.dma_start(outT[ts(ct, 128), hsl], o_sb[:])

    nc.compile()
    return nc


def _const_inputs():
    cb = np.zeros((128, 12, 128), dtype=np.float32)
    # 0: identity
    cb[:, 0, :] = np.eye(128, dtype=np.float32)
    # 1: pswap  P[c, m] = 1 iff c == swap(m), swap = +-32 within 64-block
    m = np.arange(128)
    sw = np.where((m % 64) < 32, m + 32, m - 32)
    cb[sw, 1, m] = 1.0
    # 2: Tc diag mask (keep qcol >= kpos), 3: Tw window mask (keep qcol <= kpos)
    p = np.arange(128)[:, None]
    c = np.arange(128)[None, :]
    cb[:, 2, :] = np.where(c >= p, 0.0, NEG)
    cb[:, 3, :] = np.where(c <= p, 0.0, NEG)
    # 4: cols 0:4 = stationary for even r (out rows 0:2 of msq4),
    #    cols 4:8 = odd r (out rows 2:4), cols 32:34 indq2 (k)
    cb[0:64, 4, 0] = 1.0 / D
    cb[64:128, 4, 1] = 1.0 / D
    cb[0:64, 4, 6] = 1.0 / D
    cb[64:128, 4, 7] = 1.0 / D
    cb[0:64, 4, 32] = 1.0 / D
    cb[64:128, 4, 33] = 1.0 / D
    # 5..8: ind014 per r: rsq4 row (2*(r%2)+gg) -> out gg rows, val 1/QK_SCALE
    for r in range(4):
        i = r % 2
        cb[2 * i, 5 + r, 0:64] = 1.0 / QK_SCALE
        cb[2 * i + 1, 5 + r, 64:128] = 1.0 / QK_SCALE
    # 9: indbk rows 0:2 (1/1.2), 10: indbg rows 0:2 (1.0), 11: ones row 0
    cb[0, 9, 0:64] = 1.0 / 1.2
    cb[1, 9, 64:128] = 1.0 / 1.2
    cb[0, 10, 0:64] = 1.0
    cb[1, 10, 64:128] = 1.0
    cb[0, 11, :] = 1.0
    epsb = np.full((128, 1), EPS, dtype=np.float32)
    return dict(cb16=cb.astype(BF16), cepsb=epsb)


def _prep_core_inputs(x, ve3, cosb, sinbw, Wq, Wk, Wv, Wo, Wg, consts, b, s):
    g0, g1 = 2 * s, 2 * s + 1
    bf = lambda a: np.ascontiguousarray(a).astype(BF16)
    xt = bf(x[b].T)

    Wq4 = Wq.reshape(HKV, REP, D, C)
    wq_rows = np.concatenate([Wq4[g, r] for r in range(REP) for g in (g0, g1)],
                             axis=0)                       # (512, C)
    wq = bf(wq_rows.T)                                     # (C, 512)
    Wk3 = Wk.reshape(HKV, D, C)
    wk = bf(np.concatenate([Wk3[g0], Wk3[g1]], axis=0).T)
    Wv3 = Wv.reshape(HKV, D, C)
    wv = bf(np.concatenate([Wv3[g0], Wv3[g1]], axis=0).T)

    Wo4 = Wo.reshape(C, HKV, REP, D)
    wo_cols = np.concatenate([Wo4[:, g, r, :] for r in range(REP)
                              for g in (g0, g1)], axis=1)  # (C, 512)
    wo = bf(wo_cols.T)                                     # (512, C)

    wgn = np.zeros((16, 2), dtype=np.float32)
    wgn[0:GATE_CH, 0] = -Wg[g0]
    wgn[0:GATE_CH, 1] = -Wg[g1]

    ve4 = ve3[b].reshape(T, HKV, D)
    vet = bf(np.concatenate([ve4[:, g0, :], ve4[:, g1, :]], axis=1).T)

    d = dict(xt=xt, wq=wq, wk=wk, wv=wv, wo=wo, wgn=wgn.astype(BF16),
             vet=vet, cosb=cosb, sinbw=sinbw)
    d.update(consts)
    return d


def kernel(x, ve, cos, sin, Wq, Wk, Wv, Wo, Wg, window_size):
    from concourse.bass_utils import run_bass_kernel_spmd

    assert int(window_size) == WINDOW
    x = np.asarray(x, dtype=np.float32)
    ve = np.asarray(ve, dtype=np.float32)
    Wq = np.asarray(Wq, dtype=np.float32)
    Wk = np.asarray(Wk, dtype=np.float32)
    Wv = np.asarray(Wv, dtype=np.float32)
    Wo = np.asarray(Wo, dtype=np.float32)
    Wg = np.asarray(Wg, dtype=np.float32)
    c = np.asarray(cos, dtype=np.float32).reshape(T, D // 2)   # (T, 32)
    sn = np.asarray(sin, dtype=np.float32).reshape(T, D // 2)

    cosb = np.ascontiguousarray(np.tile(c.T, (4, 1))).astype(BF16)
    sinbw = np.ascontiguousarray(
        np.concatenate([-sn.T, sn.T, -sn.T, sn.T], axis=0)).astype(BF16)
    ve3 = 3.0 * ve
    consts = _const_inputs()
    consts["cones"] = np.ones((1, 512), dtype=BF16)

    if "nc" not in _CACHE:
        _CACHE["nc"] = _build_program()
    nc = _CACHE["nc"]

    in_maps = []
    for core in range(NCORES):
        b, s = core // 2, core % 2
        in_maps.append(_prep_core_inputs(x, ve3, cosb, sinbw,
                                         Wq, Wk, Wv, Wo, Wg, consts, b, s))

    res = run_bass_kernel_spmd(nc, in_maps, core_ids=list(range(NCORES)))
    out = np.empty((B, T, C), dtype=np.float32)
    for b in range(B):
        acc = (res.results[2 * b]["out_t"].astype(np.float32)
               + res.results[2 * b + 1]["out_t"].astype(np.float32))
        out[b] = acc.T
    return out
